# revision 43
# baseline (speedup 1.0000x reference)
"""Trainium2 Bass kernel for nn_DendriticANN.

Network (reference.py):
    h = BN(leaky(x @ W_in.T + b_in))                       [B, H]
    for l in range(L):
        xn   = h / max(||h||_row, 1e-12)                   row-wise L2 normalize
        dend = leaky(einsum('bi,ndi->bnd', xn, Wd[l]))     [B, H, D]
        out  = leaky(einsum('bnd,nd->bn', dend, soma[l]))  [B, H]
        h    = BN(leaky(out))
    y = h @ W_out.T + b_out                                [B, OUT]

Sharding: data-parallel over batch across 8 cores (B=2048 -> 256 rows/core),
all parameters replicated.  Everything on-chip uses a [features, batch]
layout so BatchNorm reductions are free-axis native and layer outputs feed
the next layer's matmul K-tiles without any transposes.  BatchNorm batch
stats are combined with one 4 KB AllGather per BN (3 total).

The dendritic einsum is a plain matmul [B,H] @ [H, H*D] with the weight
columns ordered so that a 128-row output tile holds one dendrite index d for
128 neurons, and neuron blocks {0,1} fully precede blocks {2,3} (the BN
stats for the first half of the features can then be computed while the
last weight chunks are still streaming).  soma is folded into the weight
columns on host (soma*leaky(v) == Prelu(c*soma*v, alpha) with (c,alpha) =
(1, 0.01) for soma>0 and (0.01, 100) for soma<0, plus a x32 scale that
BatchNorm absorbs - eps scaled to match), so the whole soma stage is
per-partition-alpha Prelu ACTs out of PSUM plus one wide fp16 DVE
accumulate per tile pair.

Matmul operands are float16 (10-bit mantissa matches the PE's fp32r/TF32
internal precision at half the HBM traffic); PSUM accumulation is fp32.

Workaround: this walrus build rejects instructions carrying more than one
sync wait ("Too many sync wait commands"), but Tile's wait assignment
attaches one wait per producer semaphore.  Before every compile we rewrite
the BIR JSON, moving excess waits onto same-engine NoOps inserted right
before the owning instruction.
"""

import json

import numpy as np

import concourse.bass as bass
import concourse.mybir as mybir
import concourse.tile as tile
from concourse.bass_utils import run_bass_kernel_spmd

# ---------------------------------------------------------------- problem dims
N_CORES = 8
B, IN, H, D, OUT, L = 2048, 1024, 512, 32, 10, 2
BL = B // N_CORES            # 256 batch rows per core
ND = H * D                   # 16384 dendrite columns per layer
NG = H // 128                # 4 feature groups of 128
KD = H // 128                # 4 K-tiles for the dendritic matmul
KIN = IN // 128              # 8 K-tiles for the input matmul
BN_EPS = 1e-5
SLOPE = 0.01
FOLD_SCALE = 32.0
F32 = mybir.dt.float32
F16 = mybir.dt.float16

WCOLS = 2048                 # weight chunk: [128, KD, WCOLS]
NCHUNK = ND // WCOLS         # 8 column chunks per layer
TPC = WCOLS // 128           # 16 nd-tiles per chunk

# ------------------------------------------------- walrus 1-wait workaround


_patch_state = {"installed": False, "counter": 0}


def _split_excess_waits(bir_json):
    m = json.loads(bir_json)
    moved = 0
    for func in m.get("functions", []):
        for blk in func.get("blocks", []):
            new_insts = []
            for inst in blk.get("instructions", []):
                si = inst.get("sync_info") or {}
                waits = si.get("on_wait") or []
                if len(waits) > 1:
                    for w in waits[:-1]:
                        _patch_state["counter"] += 1
                        new_insts.append({
                            "opcode": "NoOp",
                            "name": f"I-waitsplit-{_patch_state['counter']}",
                            "engine": inst.get("engine", "SP"),
                            "ins": [],
                            "outs": [],
                            "debug": inst.get("debug", 0),
                            "sync_info": {"on_wait": [w], "on_update": []},
                        })
                        moved += 1
                    si["on_wait"] = [waits[-1]]
                    inst["sync_info"] = si
                new_insts.append(inst)
            blk["instructions"] = new_insts
    return json.dumps(m).encode(), moved


def _install_compile_patch():
    if _patch_state["installed"]:
        return
    _patch_state["installed"] = True
    import concourse.bass_utils as bass_utils
    import concourse.bass2jax as bass2jax

    orig = bass_utils.compile_bir_kernel

    def patched(bir_json, tmpdir, neff_name="file.neff"):
        if isinstance(bir_json, str):
            bir_json = bir_json.encode()
        bir_json, _ = _split_excess_waits(bir_json)
        return orig(bir_json, tmpdir, neff_name)

    bass_utils.compile_bir_kernel = patched
    bass2jax.compile_bir_kernel = patched


_install_compile_patch()

# ------------------------------------------------------------------ bass build


def _bn_affine_batched(nc, vec, stats_g, inv_b, eps, w=NG):
    """BN affine for w groups at once: scale_all, bias_all [128, w]."""
    mean = vec.tile([128, w], F32, tag="bn_mean", name=f"bn_mean{w}")
    ex2e = vec.tile([128, w], F32, tag="bn_ex2e", name=f"bn_ex2e{w}")
    nc.vector.tensor_scalar_mul(mean[:], stats_g[:, 0:2 * w:2], inv_b)
    # ex2 + eps in one two-scalar-op instruction
    nc.vector.tensor_scalar(ex2e[:], stats_g[:, 1:2 * w:2], inv_b, eps,
                            op0=mybir.AluOpType.mult, op1=mybir.AluOpType.add)
    msq = vec.tile([128, w], F32, tag="bn_msq", name=f"bn_msq{w}")
    nc.vector.tensor_tensor(msq[:], mean[:], mean[:], mybir.AluOpType.mult)
    vare = vec.tile([128, w], F32, tag="bn_vare", name=f"bn_vare{w}")
    nc.vector.tensor_tensor(vare[:], ex2e[:], msq[:], mybir.AluOpType.subtract)
    denom = vec.tile([128, w], F32, tag="bn_denom", name=f"bn_denom{w}")
    nc.scalar.activation(denom[:], vare[:], mybir.ActivationFunctionType.Sqrt)
    scale = vec.tile([128, w], F32, tag="bn_scale", name=f"bn_scale{w}")
    nc.vector.reciprocal(scale[:], denom[:])
    bias = vec.tile([128, w], F32, tag="bn_bias", name=f"bn_bias{w}")
    # bias = -mean * scale
    nc.vector.scalar_tensor_tensor(bias[:], mean[:], -1.0, scale[:],
                                   op0=mybir.AluOpType.mult,
                                   op1=mybir.AluOpType.mult)
    return scale, bias


def build_nc():
    nc = bass.Bass(num_devices=N_CORES)

    # host-packed layouts (partition dim first, DMA-contiguous)
    xTp = nc.dram_tensor("xTp", [128, KIN, BL], F16, kind="ExternalInput")
    w_inp = nc.dram_tensor("w_inp", [128, KIN, H], F16, kind="ExternalInput")
    b_inp = nc.dram_tensor("b_inp", [128, NG], F32, kind="ExternalInput")
    # wd: [L, NCHUNK, KD, 128, WCOLS] (chunk-contiguous)
    wdp = nc.dram_tensor("wdp", [L, NCHUNK, KD, 128, WCOLS], F16,
                         kind="ExternalInput")
    soma_d = nc.dram_tensor("soma", [L, 128, NG * D], F32,
                            kind="ExternalInput")  # prelu alpha table
    w_outp = nc.dram_tensor("w_outp", [128, NG, OUT], F16, kind="ExternalInput")
    b_out = nc.dram_tensor("b_out", [OUT, 1], F32, kind="ExternalInput")
    ones_col_d = nc.dram_tensor("ones_col", [128, 1], F16, kind="ExternalInput")
    ones_row_d = nc.dram_tensor("ones_row", [1, 128], F16, kind="ExternalInput")
    y = nc.dram_tensor("y", [OUT, BL], F32, kind="ExternalOutput")

    inv_b = 1.0 / B
    Lrelu = mybir.ActivationFunctionType.Lrelu
    Prelu = mybir.ActivationFunctionType.Prelu
    Ident = mybir.ActivationFunctionType.Identity

    with tile.TileContext(nc) as tc:
        with (
            tc.tile_pool(name="const", bufs=1) as constp,
            tc.tile_pool(name="wstream", bufs=7) as wstream,
            tc.tile_pool(name="acts", bufs=3) as acts,
            tc.tile_pool(name="work", bufs=10) as work,
            tc.tile_pool(name="vec", bufs=4) as vec,
            tc.tile_pool(name="psum_d", bufs=8, space="PSUM") as psum_d_p,
            tc.tile_pool(name="dram", bufs=2 * 3, space="DRAM") as dramp,
        ):
            # ---------------- stage-0 inputs first (earliest PE start)
            xT_sb = constp.tile([128, KIN, BL], F16)
            w_in_sb = constp.tile([128, KIN, H], F16)
            kh = KIN // 2
            # weights-first halves: k0-3 matmuls of every group run while
            # the second halves stream
            nc.sync.dma_start(w_in_sb[:, 0:kh, :], w_inp[:, 0:kh, :])
            nc.sync.dma_start(xT_sb[:, 0:kh, :], xTp[:, 0:kh, :])
            nc.sync.dma_start(w_in_sb[:, kh:, :], w_inp[:, kh:, :])
            nc.sync.dma_start(xT_sb[:, kh:, :], xTp[:, kh:, :])
            b_in_sb = constp.tile([128, NG], F32)
            nc.sync.dma_start(b_in_sb[:], b_inp[:])

            # remaining constants (used later; behind stage-0 in DMA queue)
            ones_col = constp.tile([128, 1], F16)
            nc.sync.dma_start(ones_col[:], ones_col_d[:])
            ones_row = constp.tile([1, 128], F16)
            nc.sync.dma_start(ones_row[:], ones_row_d[:])
            b_out_sb = constp.tile([OUT, 1], F32)
            nc.sync.dma_start(b_out_sb[:], b_out[:])
            w_out_sb = constp.tile([128, NG, OUT], F16)
            nc.sync.dma_start(w_out_sb[:], w_outp[:])
            soma_tiles = {}
            for l in range(L):
                t = constp.tile([128, NG * D], F32, tag=f"soma_{l}")
                nc.sync.dma_start(t[:], soma_d[l])
                soma_tiles[l] = t

            def _xchg_start(cols_ap, tag):
                """AllGather a slice of the local stats across cores."""
                w = cols_ap.shape[-1]
                st_in = dramp.tile([128, w], F32, tag=f"st_in_{tag}",
                                   name=f"st_in_{tag}")
                st_out = dramp.tile([N_CORES, 128, w], F32,
                                    tag=f"st_out_{tag}", name=f"st_out_{tag}")
                nc.sync.dma_start(st_in[:], cols_ap)
                nc.gpsimd.collective_compute(
                    "AllGather", mybir.AluOpType.bypass,
                    replica_groups=[list(range(N_CORES))],
                    ins=[st_in.opt()], outs=[st_out.opt()],
                )
                return st_out

            def _xchg_finish(st_out, w, tag):
                """Load the gathered stats and reduce over cores."""
                stats_all = vec.tile([128, N_CORES * w], F32,
                                     tag=f"stats_all_{tag}",
                                     name=f"stats_all_{tag}")
                nc.sync.dma_start(
                    stats_all[:].rearrange("p (r c) -> p r c", r=N_CORES),
                    st_out[:].rearrange("r p c -> p r c"))
                stats_g = vec.tile([128, w], F32, tag=f"stats_g_{tag}",
                                   name=f"stats_g_{tag}")
                nc.vector.tensor_reduce(
                    stats_g[:],
                    stats_all[:].rearrange("p (r c) -> p c r", r=N_CORES),
                    mybir.AxisListType.X, mybir.AluOpType.add)
                return stats_g

            # ---------------- per-BN-stage pipeline (stage 0 + L layers)
            h_tiles = None    # rhs K-tiles for the next stage's matmul

            for stage in range(L + 1):
                stats_sb = vec.tile([128, 2 * NG], F32, tag="stats")
                lq_tiles = []

                if stage == 0:
                    # input layer: psum[g] = sum_k w_in[k,g].T @ xT[k]
                    for g in range(NG):
                        ps = psum_d_p.tile([128, BL], F32, tag="psum_d")
                        for k in range(KIN):
                            nc.tensor.matmul(
                                ps[:], w_in_sb[:, k, 128 * g:128 * (g + 1)],
                                xT_sb[:, k, :],
                                start=(k == 0), stop=(k == KIN - 1))
                        lq = acts.tile([128, BL], F16, tag=f"lq{g}")
                        nc.scalar.activation(
                            lq[:], ps[:], Lrelu,
                            bias=b_in_sb[:, g:g + 1], alpha=SLOPE,
                            accum_out=stats_sb[:, 2 * g:2 * g + 1])
                        lq_tiles.append(lq)
                        # sumsq for BN var: DVE pipelines with the next
                        # group's ACT lq; the last group is tail-critical so
                        # its Square runs on ACT in parallel with DVE
                        sq = work.tile([128, BL], F16, tag="junk")
                        if g == NG - 1:
                            nc.scalar.activation(
                                sq[:], lq[:],
                                mybir.ActivationFunctionType.Square,
                                accum_out=stats_sb[:, 2 * g + 1:2 * g + 2])
                        else:
                            nc.vector.tensor_tensor(sq[:], lq[:], lq[:],
                                                    mybir.AluOpType.mult)
                            nc.vector.tensor_reduce(
                                stats_sb[:, 2 * g + 1:2 * g + 2], sq[:],
                                mybir.AxisListType.X, mybir.AluOpType.add)
                else:
                    l = stage - 1
                    # dendritic matmul on the UNNORMALIZED h: since Prelu is
                    # positively homogeneous, the row L2-normalization factor
                    # rinv_b is applied exactly at the lq stage below (one DVE
                    # scalar_tensor_tensor per group) instead of gating the
                    # matmuls at the BN boundary.  The rinv chain itself is
                    # emitted after chunk 0 so it overlaps the matmul stream.
                    h_in = h_tiles
                    acc_all = acts.tile([128, NG * BL], F16, tag="acc_all")
                    # hsq first in DVE program order: DVE is idle right after
                    # the BN boundary, so these finish immediately and the
                    # row-norm chain below never blocks the PE
                    hsq_tiles = []
                    for g in range(NG):
                        hsq = work.tile([128, BL], F16, tag="junk")
                        nc.vector.tensor_tensor(
                            hsq[:], h_in[g][:], h_in[g][:],
                            mybir.AluOpType.mult)
                        hsq_tiles.append(hsq)
                    nc.vector.memset(acc_all[:], 0.0)
                    rinvb = acts.tile([128, BL], F16, tag="rinvb")
                    for cc in range(NCHUNK):
                        w = wstream.tile([128, KD, WCOLS], F16, tag="wchunk")
                        for k in range(KD):
                            nc.sync.dma_start(w[:, k, :], wdp[l, cc, k])
                        for tp in range(TPC // 2):
                            ps = psum_d_p.tile([128, 2 * BL], F32,
                                               tag="psum_d")
                            for half in range(2):
                                tt = 2 * tp + half
                                for k in range(KD):
                                    nc.tensor.matmul(
                                        ps[:, BL * half:BL * (half + 1)],
                                        w[:, k, 128 * tt:128 * (tt + 1)],
                                        h_in[k][:],
                                        start=(k == 0), stop=(k == KD - 1))
                            # global tile index -> (block, d, nb-within-block)
                            t_glob = cc * TPC + 2 * tp
                            blk, rem = divmod(t_glob, 64)
                            d_idx, nbb = divmod(rem, 2)
                            nb = 2 * blk + nbb
                            sm = work.tile([128, 2 * BL], F16, tag="sm")
                            for half in range(2):
                                acol = soma_tiles[l][
                                    :, (nb + half) * D + d_idx:
                                       (nb + half) * D + d_idx + 1]
                                nc.scalar.activation(
                                    sm[:, BL * half:BL * (half + 1)],
                                    ps[:, BL * half:BL * (half + 1)],
                                    Prelu, alpha=acol)
                            accs = acc_all[:, nb * BL:(nb + 2) * BL]
                            nc.vector.tensor_tensor(
                                accs, accs, sm[:], mybir.AluOpType.add)
                        if cc == 0:
                            # row L2 norm of the layer input, off the critical
                            # path: rinv[b] = 1/sqrt(max(sum_f h^2, eps))
                            ps_r = psum_d_p.tile([1, BL], F32, tag="psum_d")
                            for g in range(NG):
                                nc.tensor.matmul(ps_r[:], ones_col[:],
                                                 hsq_tiles[g][:],
                                                 start=(g == 0),
                                                 stop=(g == NG - 1))
                            ssq = vec.tile([1, BL], F32, tag="ssq")
                            nc.vector.tensor_scalar_max(ssq[:], ps_r[:], 1e-24)
                            rnorm = vec.tile([1, BL], F32, tag="rnorm")
                            nc.scalar.activation(
                                rnorm[:], ssq[:],
                                mybir.ActivationFunctionType.Sqrt)
                            rinv = vec.tile([1, BL], F16, tag="rinv")
                            with nc.allow_low_precision(
                                    reason="rinv rounding is benign"):
                                nc.vector.reciprocal(rinv[:], rnorm[:])
                        if cc == 1:
                            # broadcast rinv across partitions; rinv is long
                            # done by the time the PE drains chunk 1
                            ps_b = psum_d_p.tile([128, BL], F32, tag="psum_d")
                            nc.tensor.matmul(ps_b[:], ones_row[:], rinv[:],
                                             start=True, stop=True)
                            nc.vector.tensor_scalar_mul(rinvb[:], ps_b[:],
                                                         1.0)
                        if cc == NCHUNK // 2 - 1 or cc == NCHUNK - 1:
                            # neuron blocks for this half are complete:
                            # fold in lq + stats while the rest streams
                            for g in (0, 1) if cc == NCHUNK // 2 - 1 else (2, 3):
                                lq2 = work.tile([128, BL], F16, tag="lq2")
                                # reference applies leaky twice here
                                nc.scalar.activation(
                                    lq2[:], acc_all[:, g * BL:(g + 1) * BL],
                                    Prelu, alpha=SLOPE * SLOPE)
                                lq = acts.tile([128, BL], F16, tag=f"lq{g}")
                                # lq = lq2 * rinv_b (exact row L2 normalize)
                                nc.vector.scalar_tensor_tensor(
                                    lq[:], lq2[:], 1.0, rinvb[:],
                                    op0=mybir.AluOpType.mult,
                                    op1=mybir.AluOpType.mult)
                                lq_tiles.append(lq)
                                nc.vector.tensor_reduce(
                                    stats_sb[:, 2 * g:2 * g + 1], lq[:],
                                    mybir.AxisListType.X, mybir.AluOpType.add)
                                sq = work.tile([128, BL], F16, tag="junk")
                                if g >= 2:
                                    # tail-critical: ACT Square so the stat
                                    # chains run on ACT and DVE in parallel
                                    nc.scalar.activation(
                                        sq[:], lq[:],
                                        mybir.ActivationFunctionType.Square,
                                        accum_out=stats_sb[:,
                                                           2 * g + 1:
                                                           2 * g + 2])
                                else:
                                    # mid-layer: keep it off ACT, which is
                                    # the near-saturated engine there
                                    nc.vector.tensor_tensor(
                                        sq[:], lq[:], lq[:],
                                        mybir.AluOpType.mult)
                                    nc.vector.tensor_reduce(
                                        stats_sb[:, 2 * g + 1:2 * g + 2],
                                        sq[:], mybir.AxisListType.X,
                                        mybir.AluOpType.add)
                            if cc == NCHUNK // 2 - 1:
                                # launch the g01 AllGather now: it completes
                                # while the g23 chunks are still streaming.
                                # Only Pool/SP instructions here - anything
                                # that WAITS on its result would head-of-line
                                # block ACT/DVE for the rest of the layer.
                                st_a = _xchg_start(stats_sb[:, 0:NG], "a")

                # ---- finish the stat exchange(s) and apply BN.
                # Layer stages launched the g01 AllGather mid-layer (see
                # _xchg_start at cc==3); here we launch the g23 one, then
                # consume both.  Stage 0 does a single full-width exchange.
                eps = BN_EPS if stage == 0 else BN_EPS * FOLD_SCALE * FOLD_SCALE
                h_tiles = []
                if stage == 0:
                    st0 = _xchg_start(stats_sb[:, 0:2 * NG], "f")
                    stats_g = _xchg_finish(st0, 2 * NG, "f")
                    scale_all, bias_all = _bn_affine_batched(
                        nc, vec, stats_g, inv_b, eps, NG)
                    for g in range(NG):
                        h = acts.tile([128, BL], F16, tag=f"h{g}")
                        nc.scalar.activation(h[:], lq_tiles[g][:], Ident,
                                             bias=bias_all[:, g:g + 1],
                                             scale=scale_all[:, g:g + 1])
                        h_tiles.append(h)
                else:
                    st_b = _xchg_start(stats_sb[:, NG:2 * NG], "b")
                    # g01: the AllGather completed during the layer
                    stats_ga = _xchg_finish(st_a, NG, "a")
                    sc_a, bi_a = _bn_affine_batched(nc, vec, stats_ga,
                                                    inv_b, eps, NG // 2)
                    for g in (0, 1):
                        h = acts.tile([128, BL], F16, tag=f"h{g}")
                        nc.scalar.activation(h[:], lq_tiles[g][:], Ident,
                                             bias=bi_a[:, g:g + 1],
                                             scale=sc_a[:, g:g + 1])
                        h_tiles.append(h)
                    stats_gb = _xchg_finish(st_b, NG, "b")
                    sc_b, bi_b = _bn_affine_batched(nc, vec, stats_gb,
                                                    inv_b, eps, NG // 2)
                    for g in (2, 3):
                        h = acts.tile([128, BL], F16, tag=f"h{g}")
                        nc.scalar.activation(h[:], lq_tiles[g][:], Ident,
                                             bias=bi_b[:, g - 2:g - 1],
                                             scale=sc_b[:, g - 2:g - 1])
                        h_tiles.append(h)

            # ---------------- output layer: y = h @ W_out.T + b_out
            ps_y = psum_d_p.tile([OUT, BL], F32, tag="psum_d")
            for g in range(NG):
                nc.tensor.matmul(ps_y[:], w_out_sb[:, g, :],
                                 h_tiles[g][:], start=(g == 0),
                                 stop=(g == NG - 1))
            y_sb = work.tile([OUT, BL], F32, tag="ld")
            nc.scalar.activation(y_sb[:], ps_y[:], Ident, bias=b_out_sb[:])
            nc.sync.dma_start(y[:], y_sb[:])

    return nc


# ------------------------------------------------------------------ host side

_cache = {}


def _get_nc():
    if "nc" not in _cache:
        _cache["nc"] = build_nc()
    return _cache["nc"]


def make_in_maps(x, W_in, b_in, Wd, soma, W_out, b_out):
    mm_np = np.float16
    xT = np.ascontiguousarray(x.T.astype(mm_np))            # [IN, B]
    xTp_full = xT.reshape(KIN, 128, B).transpose(1, 0, 2)   # [128, KIN, B]
    w_inp = np.ascontiguousarray(
        W_in.T.astype(mm_np).reshape(KIN, 128, H).transpose(1, 0, 2))
    b_inp = np.ascontiguousarray(
        b_in.reshape(NG, 128).T.astype(np.float32))

    # Fold the soma weights into the dendritic weight columns:
    #   soma*leaky(v) == Prelu(c*soma*v, alpha) with (c, alpha) =
    #   (1, 0.01) for soma>0 and (0.01, 100) for soma<0.
    # A further x32 keeps the folded fp16 weights out of subnormal range;
    # BatchNorm makes the network exactly invariant to this positive scale.
    soma_c = np.where(soma > 0, soma, SLOPE * soma) * FOLD_SCALE
    fold = soma_c.transpose(0, 2, 1)[:, None, :, :]         # [L, 1, D, H]
    wd_f = Wd.transpose(0, 3, 2, 1) * fold                  # [L, i, D, n]
    # column order: neuron block pair {0,1} all-d first, then {2,3}:
    #   flat col = blk*8192 + d*256 + n_within_256
    wd_b = wd_f.reshape(L, H, D, 2, 256).transpose(0, 1, 3, 2, 4)
    wd2 = wd_b.reshape(L, H, ND).astype(mm_np)              # [L, K, ND]
    # chunk-pack: [L, NCHUNK, KD, 128, WCOLS]
    wdp = np.ascontiguousarray(
        wd2.reshape(L, KD, 128, NCHUNK, WCOLS).transpose(0, 3, 1, 2, 4))

    alpha = np.where(soma > 0, SLOPE, 1.0 / SLOPE).astype(np.float32)
    soma2 = np.ascontiguousarray(
        alpha.reshape(L, NG, 128, D).transpose(0, 2, 1, 3).reshape(
            L, 128, NG * D))
    w_outp = np.ascontiguousarray(
        W_out.T.astype(mm_np).reshape(NG, 128, OUT).transpose(1, 0, 2))
    common = dict(
        w_inp=w_inp,
        b_inp=b_inp,
        wdp=wdp,
        soma=soma2,
        w_outp=w_outp,
        b_out=np.ascontiguousarray(b_out.reshape(OUT, 1), dtype=np.float32),
        ones_col=np.ones((128, 1), dtype=mm_np),
        ones_row=np.ones((1, 128), dtype=mm_np),
    )
    in_maps = []
    for c in range(N_CORES):
        m = dict(common)
        m["xTp"] = np.ascontiguousarray(xTp_full[:, :, BL * c:BL * (c + 1)])
        in_maps.append(m)
    return in_maps


def kernel(x, W_in, b_in, Wd, soma, W_out, b_out):
    in_maps = make_in_maps(np.asarray(x, dtype=np.float32),
                           np.asarray(W_in), np.asarray(b_in),
                           np.asarray(Wd), np.asarray(soma),
                           np.asarray(W_out), np.asarray(b_out))
    nc = _get_nc()
    res = run_bass_kernel_spmd(nc, in_maps, core_ids=list(range(N_CORES)))
    y = np.concatenate([r["y"] for r in res.results], axis=1)  # [OUT, B]
    return np.ascontiguousarray(y.T, dtype=np.float32)


if __name__ == "__main__":
    rng = np.random.default_rng(0)
    x = rng.standard_normal((B, IN), dtype=np.float32)
    W_in = (rng.standard_normal((H, IN), dtype=np.float32) / np.sqrt(IN))
    b_in_a = np.zeros(H, np.float32)
    Wd_a = rng.standard_normal((L, H, D, H), dtype=np.float32) * 0.1
    soma_a = rng.standard_normal((L, H, D), dtype=np.float32) * 0.1
    W_out = rng.standard_normal((OUT, H), dtype=np.float32) / np.sqrt(H)
    b_out_a = np.zeros(OUT, np.float32)
    y = kernel(x=x, W_in=W_in, b_in=b_in_a, Wd=Wd_a, soma=soma_a,
               W_out=W_out, b_out=b_out_a)
    print("kernel output:", y.shape, y.dtype, float(np.abs(y).max()))
